# revision 2
# baseline (speedup 1.0000x reference)
"""GATv2-Salt device kernel: 8 NeuronCores via Bass (see gat_core.py)."""
import sys
for p in ('/opt/trn_rl_repo', '/root/problem'):
    if p not in sys.path:
        sys.path.insert(0, p)
import numpy as np
import gat_core

_CACHE = {}


def kernel(**inputs):
    from concourse.bass_utils import run_bass_kernel_spmd
    import time
    inputs = {k: np.asarray(v) for k, v in inputs.items()}
    cfg = gat_core.make_cfg()
    t0 = time.time()
    in_maps, _ = gat_core.prep(inputs, cfg)
    t1 = time.time()
    if "nc" not in _CACHE:
        _CACHE["nc"] = gat_core.build(cfg)
    t2 = time.time()
    res = run_bass_kernel_spmd(_CACHE["nc"], in_maps,
                               core_ids=list(range(cfg["NC"])))
    t3 = time.time()
    kernel.timings = dict(prep=t1 - t0, build=t2 - t1, run=t3 - t2)
    y = np.concatenate([res.results[c]["y"] for c in range(cfg["NC"])],
                       axis=0)[:cfg["B"]]
    return np.ascontiguousarray(y, np.float32)
